# revision 19
# baseline (speedup 1.0000x reference)
"""ExpDecayRNN Trainium2 kernel.

Data-parallel over batch: B=64 split across 8 NeuronCores (8 batches/core).
All arithmetic runs on-device; the host only reshapes/transposes/shards.

Layouts (per core, B=8 local batches):
  phase 1: per-batch [D=128 partitions, T free] tiles.
  phase 2: transposed hidden state h.T as [128 partitions, 2*B] tile
           (H=256 split into two 128-row halves side by side in the free dim).
  gi scratch (DRAM): [T, 6, B, 128]  (t, gate-chunk, batch, gate-row%128)
  decay (SBUF-resident): [128, T*16] with free index = 16*t + 8*half + b
  hidden accum chunk: [128, TC*16] with free index = 16*tt + 8*half + b
"""

import numpy as np

import concourse.bass as bass
import concourse.mybir as mybir
import concourse.tile as tile

F32 = mybir.dt.float32
AF = mybir.ActivationFunctionType
OP = mybir.AluOpType

B = 8          # batches per core
D = 128
H = 256
GATE_CHUNKS = 6  # 3H/128
N_CORES = 8


def split_waits(nc, limit=1):
    """This walrus build accepts at most one sync wait per instruction;
    hoist extras onto single-wait NoOps preceding the instruction."""
    for bb in nc.main_func.blocks:
        new_insts = []
        for ins in bb.instructions:
            si = ins.sync_info
            waits = list(si.on_wait) if si is not None and si.on_wait else []
            if len(waits) > limit:
                extras, keep = waits[:-limit], waits[-limit:]
                for i, w in enumerate(extras):
                    nop = mybir.InstNoOp(name=f"{ins.name}-wsplit{i}", ins=[], outs=[])
                    nop.engine = ins.engine
                    nop.sync_info = mybir.SyncInfo(on_wait=[w], on_update=[])
                    nc.register_instruction(nop, overwrite=True)
                    new_insts.append(nop)
                si.on_wait = keep
            new_insts.append(ins)
        bb.instructions = new_insts


def build_nc(T=1024, TC=128, passes=1):
    assert T % TC == 0
    nc = bass.Bass()

    # ---- I/O ----
    xT = nc.dram_tensor("xT", [B, D, T], F32, kind="ExternalInput")
    maskT = nc.dram_tensor("maskT", [B, D, T], F32, kind="ExternalInput")
    tsteps = nc.dram_tensor("tsteps", [B, T], F32, kind="ExternalInput")
    tinit = nc.dram_tensor("tinit", [B, 1], F32, kind="ExternalInput")
    wihT = nc.dram_tensor("wihT", [D, 3 * H], F32, kind="ExternalInput")
    whhT = nc.dram_tensor("whhT", [H, 3 * H], F32, kind="ExternalInput")
    idwT = nc.dram_tensor("idwT", [D, D], F32, kind="ExternalInput")
    idb = nc.dram_tensor("idb", [D, 1], F32, kind="ExternalInput")
    b_rz_ih = nc.dram_tensor("b_rz_ih", [128, 4], F32, kind="ExternalInput")
    b_rz_hh = nc.dram_tensor("b_rz_hh", [128, 4], F32, kind="ExternalInput")
    b_n_ih = nc.dram_tensor("b_n_ih", [128, 2], F32, kind="ExternalInput")
    b_n_hh = nc.dram_tensor("b_n_hh", [128, 2], F32, kind="ExternalInput")
    hdw = nc.dram_tensor("hdw", [128, 2], F32, kind="ExternalInput")
    hdb = nc.dram_tensor("hdb", [128, 2], F32, kind="ExternalInput")

    hidden = nc.dram_tensor("hidden", [B, T, H], F32, kind="ExternalOutput")
    hlast = nc.dram_tensor("hlast", [B, H], F32, kind="ExternalOutput")

    gi_scr = nc.dram_tensor("gi_scr", [T, GATE_CHUNKS, B, 128], F32)
    dec_scr = nc.dram_tensor("dec_scr", [T, 2, B, 128], F32)
    td_scr = nc.dram_tensor("td_scr", [B, T], F32)
    tdl_scr = nc.dram_tensor("tdl_scr", [B, T], F32)

    with tile.TileContext(nc) as tc:
        with (
            tc.tile_pool(name="consts", bufs=1) as consts,
            tc.tile_pool(name="persist", bufs=1) as persist,
            tc.tile_pool(name="bwork", bufs=1) as bwork,
            tc.tile_pool(name="psum", bufs=2, space="PSUM") as psum,
            tc.tile_pool(name="psg", bufs=2, space="PSUM") as psg,
            tc.tile_pool(name="gich", bufs=2) as gich,
            tc.tile_pool(name="decch", bufs=2) as decch,
            tc.tile_pool(name="accch", bufs=2) as accch,
            tc.tile_pool(name="step", bufs=3) as steppool,
        ):
          for _pass in range(passes):
            # ---- constants / weights resident in SBUF ----
            zeros = consts.tile([128, T], F32)
            nc.vector.memset(zeros[:], 0.0)
            ones128 = consts.tile([128, 1], F32)
            nc.vector.memset(ones128[:], 1.0)

            wih_sb = consts.tile([128, 3 * H], F32)
            nc.sync.dma_start(wih_sb[:], wihT[:, :])
            whh0 = consts.tile([128, 3 * H], F32)
            nc.sync.dma_start(whh0[:], whhT[0:128, :])
            whh1 = consts.tile([128, 3 * H], F32)
            nc.sync.dma_start(whh1[:], whhT[128:256, :])
            idw_sb = consts.tile([128, D], F32)
            nc.sync.dma_start(idw_sb[:], idwT[:, :])

            idb_sb = consts.tile([128, 1], F32)
            nc.sync.dma_start(idb_sb[:], idb[:, :])
            brzi = consts.tile([128, 4], F32)
            nc.sync.dma_start(brzi[:], b_rz_ih[:, :])
            brzh = consts.tile([128, 4], F32)
            nc.sync.dma_start(brzh[:], b_rz_hh[:, :])
            bni = consts.tile([128, 2], F32)
            nc.sync.dma_start(bni[:], b_n_ih[:, :])
            bnh = consts.tile([128, 2], F32)
            nc.sync.dma_start(bnh[:], b_n_hh[:, :])
            hdw_sb = consts.tile([128, 2], F32)
            nc.sync.dma_start(hdw_sb[:], hdw[:, :])
            hdb_sb = consts.tile([128, 2], F32)
            nc.sync.dma_start(hdb_sb[:], hdb[:, :])

            # rz bias (ih+hh) for phase-B fold
            brz = consts.tile([128, 4], F32)
            nc.vector.tensor_add(brz[:], brzi[:], brzh[:])
            # negated hidden-decay params for fused exp
            nhdw = consts.tile([128, 2], F32)
            nc.vector.tensor_scalar_mul(nhdw[:], hdw_sb[:], -1.0)
            nhdb = consts.tile([128, 2], F32)
            nc.vector.tensor_scalar_mul(nhdb[:], hdb_sb[:], -1.0)
            # b_hh_n expanded to [128, 16] (half-major, 8 batches each)
            bhn_exp = consts.tile([128, 16], F32)
            for h in range(2):
                nc.vector.tensor_scalar_add(
                    bhn_exp[:, 8 * h : 8 * h + 8], zeros[:, 0:8], bnh[:, h : h + 1]
                )
            # fill_w bias: pre = ic'@idwT + (idb - rowsum(idw)); exp bias = rowsum - idb
            ps_rs = psum.tile([128, 1], F32, tag="psfw")
            nc.tensor.matmul(ps_rs[:], idw_sb[:], ones128[:], start=True, stop=True)
            fw_ebias = consts.tile([128, 1], F32)
            nc.vector.tensor_tensor(
                out=fw_ebias[:], in0=ps_rs[:], in1=idb_sb[:], op=OP.subtract
            )

            # initial hidden state h_{-1}=0 [128, 16] (two H-halves side by side);
            # after step t the live state is the acc slice written by that step.
            state0 = persist.tile([128, 16], F32)
            nc.vector.memset(state0[:], 0.0)

            # ---- stage A: time deltas (all batches at once, [B, T] tiles) ----
            ts8 = consts.tile([B, T + 1], F32)
            nc.sync.dma_start(ts8[:, 0:1], tinit[:, :])
            nc.sync.dma_start(ts8[:, 1 : T + 1], tsteps[:, :])
            td8 = consts.tile([B, T], F32)
            nc.vector.tensor_tensor(
                out=td8[:], in0=ts8[:, 1 : T + 1], in1=ts8[:, 0:T], op=OP.subtract
            )
            tdl8 = consts.tile([B, T], F32)
            nc.vector.tensor_scalar_min(tdl8[:], td8[:], 1000.0)
            nc.scalar.activation(tdl8[:], tdl8[:], AF.Ln)
            # spill to DRAM so per-batch [128, T] broadcasts can use
            # zero-stride partition APs on the DMA source
            nc.sync.dma_start(td_scr[:, :], td8[:])
            nc.sync.dma_start(tdl_scr[:, :], tdl8[:])

            # ---- phase 1 + gi per batch ----
            for b in range(B):
                tdbc = bwork.tile([128, T], F32, tag="tdbc")
                nc.sync.dma_start(tdbc[:], bass.AP(td_scr, b * T, [[0, 128], [1, T]]))
                tdlbc = bwork.tile([128, T], F32, tag="tdlbc")
                nc.sync.dma_start(tdlbc[:], bass.AP(tdl_scr, b * T, [[0, 128], [1, T]]))

                # hidden decay for this batch -> DRAM scratch
                for h in range(2):
                    e = bwork.tile([128, T], F32, tag="dexp")
                    nc.scalar.activation(
                        e[:], tdlbc[:], AF.Exp,
                        bias=nhdb[:, h : h + 1], scale=nhdw[:, h : h + 1],
                    )
                    e2 = bwork.tile([128, T], F32, tag="dexp2")
                    nc.vector.tensor_scalar_min(e2[:], e[:], 1.0)
                    nc.sync.dma_start(
                        dec_scr[:, h, b, :].rearrange("t p -> p t"), e2[:]
                    )

                # mask tile with zero prefix column
                mT = bwork.tile([128, T + 1], F32, tag="mT")
                nc.vector.memset(mT[:, 0:1], 0.0)
                nc.sync.dma_start(mT[:, 1 : T + 1], maskT[b, :, :])
                xt = bwork.tile([128, T], F32, tag="xt")
                nc.sync.dma_start(xt[:], xT[b, :, :])

                # time-since-last-observation (pre-reset) scan:
                # v_t = m_{t-1} * v_{t-1} + td_t
                vt = bwork.tile([128, T], F32, tag="vt")
                nc.vector.tensor_tensor_scan(
                    vt[:], mT[:, 0:T], tdbc[:], 0.0, op0=OP.mult, op1=OP.add
                )
                # ic' = clip(v, 1, 1001); the -1 is folded into the fw bias
                ict = bwork.tile([128, T], F32, tag="ict")
                nc.vector.tensor_scalar(
                    ict[:], vt[:], 1.0, 1001.0, op0=OP.max, op1=OP.min
                )
                # fill weight: fw = min(1, exp(-(ic'@idwT + idb - rowsum)))
                fwt = bwork.tile([128, T], F32, tag="fwt")
                nchunks = max(1, T // 512)
                csz = min(T, 512)
                for ck in range(nchunks):
                    ps_fw = psum.tile([128, csz], F32, tag="psfw")
                    nc.tensor.matmul(
                        ps_fw[:], idw_sb[:], ict[:, ck * csz : (ck + 1) * csz],
                        start=True, stop=True,
                    )
                    nc.scalar.activation(
                        fwt[:, ck * csz : (ck + 1) * csz], ps_fw[:], AF.Exp,
                        bias=fw_ebias[:, 0:1], scale=-1.0,
                    )
                nc.vector.tensor_scalar_min(fwt[:], fwt[:], 1.0)

                # xm = x where observed else 0
                xm = bwork.tile([128, T], F32, tag="xm")
                nc.vector.tensor_copy(xm[:], xt[:])
                nc.vector.copy_predicated(xm[:], mT[:, 1 : T + 1].bitcast(mybir.dt.int32), zeros[:, 0:T])
                # forward-fill scan: f_t = m_t * f_{t-1} + xm_t
                fft = bwork.tile([128, T], F32, tag="fft")
                nc.vector.tensor_tensor_scan(
                    fft[:], mT[:, 1 : T + 1], xm[:], 0.0, op0=OP.mult, op1=OP.add
                )
                # mean = sum(xm) / max(1, sum(m))   (torch-faithful denominator)
                xsum = bwork.tile([128, 1], F32, tag="xsum")
                nc.vector.tensor_reduce(xsum[:], xm[:], axis=mybir.AxisListType.X, op=OP.add)
                msum = bwork.tile([128, 1], F32, tag="msum")
                nc.vector.tensor_reduce(
                    msum[:], mT[:, 1 : T + 1], axis=mybir.AxisListType.X, op=OP.add
                )
                nc.vector.tensor_scalar_max(msum[:], msum[:], 1.0)
                recd = bwork.tile([128, 1], F32, tag="recd")
                nc.vector.reciprocal(recd[:], msum[:])
                meant = bwork.tile([128, 1], F32, tag="meant")
                nc.vector.tensor_tensor(out=meant[:], in0=xsum[:], in1=recd[:], op=OP.mult)

                # filled = mean + fw*(ffill - mean); x_proc = where(m, filled, x)
                t1 = bwork.tile([128, T], F32, tag="t1")
                nc.vector.scalar_tensor_tensor(
                    out=t1[:], in0=fft[:], scalar=meant[:, 0:1], in1=fwt[:],
                    op0=OP.subtract, op1=OP.mult,
                )
                nc.scalar.activation(t1[:], t1[:], AF.Identity, bias=meant[:, 0:1])
                nc.vector.copy_predicated(xt[:], mT[:, 1 : T + 1].bitcast(mybir.dt.int32), t1[:])

                # gi = x_proc @ w_ih.T (+ folded biases), spilled to DRAM scratch
                for c in range(GATE_CHUNKS):
                    bias_ap = brz[:, c : c + 1] if c < 4 else bni[:, c - 4 : c - 3]
                    for ck in range(nchunks):
                        ps_gi = psum.tile([128, csz], F32, tag="psgi")
                        nc.tensor.matmul(
                            ps_gi[:],
                            wih_sb[:, 128 * c : 128 * (c + 1)],
                            xt[:, ck * csz : (ck + 1) * csz],
                            start=True, stop=True,
                        )
                        gtmp = bwork.tile([128, csz], F32, tag="gtmp")
                        nc.scalar.activation(gtmp[:], ps_gi[:], AF.Identity, bias=bias_ap)
                        nc.sync.dma_start(
                            gi_scr[ck * csz : (ck + 1) * csz, c, b, :].rearrange(
                                "t p -> p t"
                            ),
                            gtmp[:],
                        )

            # ---- phase 2: the sequential scan ----
            n_chunks = T // TC
            state = state0[:]
            for k in range(n_chunks):
                gi_sb = gich.tile([128, TC * 48], F32, tag="gi")
                nc.sync.dma_start(
                    gi_sb[:],
                    gi_scr[k * TC : (k + 1) * TC, :, :, :].rearrange(
                        "t c b p -> p (t c b)"
                    ),
                )
                dec_sb = decch.tile([128, TC * 16], F32, tag="dec")
                nc.sync.dma_start(
                    dec_sb[:],
                    dec_scr[k * TC : (k + 1) * TC, :, :, :].rearrange(
                        "t h b p -> p (t h b)"
                    ),
                )
                acc = accch.tile([128, TC * 16], F32, tag="acc")

                for tt in range(TC):
                    t = k * TC + tt
                    # h' = decay_t * h_{t-1}
                    hp = steppool.tile([128, 16], F32, tag="hp")
                    nc.vector.tensor_tensor(
                        out=hp[:], in0=state, in1=dec_sb[:, 16 * tt : 16 * tt + 16],
                        op=OP.mult,
                    )
                    # gh = h' @ w_hh.T  (12 matmuls, K=2x128, M=128, N=8)
                    ps_g = psg.tile([128, 48], F32, tag="psg")
                    for c in range(GATE_CHUNKS):
                        nc.tensor.matmul(
                            ps_g[:, 8 * c : 8 * c + 8],
                            whh0[:, 128 * c : 128 * (c + 1)],
                            hp[:, 0:8],
                            start=True, stop=False,
                        )
                        nc.tensor.matmul(
                            ps_g[:, 8 * c : 8 * c + 8],
                            whh1[:, 128 * c : 128 * (c + 1)],
                            hp[:, 8:16],
                            start=False, stop=True,
                        )
                    ps_s = psg.tile([128, 48], F32, tag="pss")
                    # pre_rz = gh_rz + gi_rz' (biases already folded into gi)
                    nc.vector.tensor_tensor(
                        out=ps_s[:, 0:32], in0=ps_g[:, 0:32],
                        in1=gi_sb[:, 48 * tt : 48 * tt + 32], op=OP.add,
                    )
                    rzt = steppool.tile([128, 32], F32, tag="rzt")
                    nc.scalar.activation(rzt[:], ps_s[:, 0:32], AF.Sigmoid)
                    # n = tanh(gi_n' + r*(gh_n + b_hh_n))
                    ut = steppool.tile([128, 16], F32, tag="ut")
                    nc.vector.tensor_tensor(
                        out=ut[:], in0=ps_g[:, 32:48], in1=bhn_exp[:], op=OP.add
                    )
                    u2 = steppool.tile([128, 16], F32, tag="u2")
                    nc.vector.tensor_tensor(out=u2[:], in0=ut[:], in1=rzt[:, 0:16], op=OP.mult)
                    nc.vector.tensor_tensor(
                        out=ps_s[:, 32:48], in0=u2[:],
                        in1=gi_sb[:, 48 * tt + 32 : 48 * tt + 48], op=OP.add,
                    )
                    nt = steppool.tile([128, 16], F32, tag="nt")
                    nc.scalar.activation(nt[:], ps_s[:, 32:48], AF.Tanh)
                    # h_t = n + z*(h' - n)
                    dt_ = steppool.tile([128, 16], F32, tag="dt")
                    nc.vector.tensor_tensor(out=dt_[:], in0=hp[:], in1=nt[:], op=OP.subtract)
                    et = steppool.tile([128, 16], F32, tag="et")
                    nc.vector.tensor_tensor(out=et[:], in0=rzt[:, 16:32], in1=dt_[:], op=OP.mult)
                    # h_t goes straight into the output accumulator; it is also
                    # the live state for step t+1.
                    state = acc[:, 16 * tt : 16 * tt + 16]
                    nc.vector.tensor_tensor(out=state, in0=nt[:], in1=et[:], op=OP.add)

                acc_r = acc[:].rearrange("p (t hh b) -> p t hh b", t=TC, hh=2)
                for b in range(B):
                    nc.sync.dma_start(
                        hidden[b, k * TC : (k + 1) * TC, :].rearrange(
                            "t (hh p) -> p t hh", p=128
                        ),
                        acc_r[:, :, :, b],
                    )

            state_r = state.rearrange("p (hh b) -> p hh b", hh=2)
            for b in range(B):
                nc.sync.dma_start(
                    hlast[b, :].rearrange("(hh p) -> p hh", p=128),
                    state_r[:, :, b],
                )

    split_waits(nc)
    return nc


def _host_prepare(inputs, T):
    """Pure layout transforms; no arithmetic beyond dtype cast of the bool mask."""
    x = np.asarray(inputs["input"], np.float32)
    mask = np.asarray(inputs["mask"]).astype(np.float32)
    time_step = np.asarray(inputs["time_step"], np.float32)
    init_time = np.asarray(inputs["init_time"], np.float32)
    w_ih = np.asarray(inputs["w_ih"], np.float32)
    w_hh = np.asarray(inputs["w_hh"], np.float32)
    b_ih = np.asarray(inputs["b_ih"], np.float32)
    b_hh = np.asarray(inputs["b_hh"], np.float32)
    idw = np.asarray(inputs["input_decay_w"], np.float32)
    idb = np.asarray(inputs["input_decay_b"], np.float32)
    hdw = np.asarray(inputs["hidden_decay_w"], np.float32)
    hdb = np.asarray(inputs["hidden_decay_b"], np.float32)

    shared = dict(
        wihT=np.ascontiguousarray(w_ih.T),
        whhT=np.ascontiguousarray(w_hh.T),
        idwT=np.ascontiguousarray(idw.T),
        idb=np.ascontiguousarray(idb.reshape(1, D).T),
        b_rz_ih=np.ascontiguousarray(b_ih[: 2 * H].reshape(4, 128).T),
        b_rz_hh=np.ascontiguousarray(b_hh[: 2 * H].reshape(4, 128).T),
        b_n_ih=np.ascontiguousarray(b_ih[2 * H :].reshape(2, 128).T),
        b_n_hh=np.ascontiguousarray(b_hh[2 * H :].reshape(2, 128).T),
        hdw=np.ascontiguousarray(hdw[:, 0].reshape(2, 128).T),
        hdb=np.ascontiguousarray(hdb.reshape(2, 128).T),
    )
    in_maps = []
    for c in range(N_CORES):
        sl = slice(c * B, (c + 1) * B)
        m = dict(shared)
        m["xT"] = np.ascontiguousarray(x[sl].transpose(0, 2, 1))
        m["maskT"] = np.ascontiguousarray(mask[sl].transpose(0, 2, 1))
        m["tsteps"] = np.ascontiguousarray(time_step[sl])
        m["tinit"] = np.ascontiguousarray(init_time[0, sl].reshape(B, 1))
        in_maps.append(m)
    return in_maps


def kernel(**inputs):
    from concourse.bass_utils import run_bass_kernel_spmd

    T = inputs["input"].shape[1]
    nc = build_nc(T=T, TC=min(128, T))
    in_maps = _host_prepare(inputs, T)
    res = run_bass_kernel_spmd(nc, in_maps, core_ids=list(range(N_CORES)))
    hidden = np.concatenate([r["hidden"] for r in res.results], axis=0)
    hlast = np.concatenate([r["hlast"] for r in res.results], axis=0)
    return hidden, hlast[None]


# revision 25
# speedup vs baseline: 3.6940x; 3.6940x over previous
"""ExpDecayRNN Trainium2 kernel (v3).

Data-parallel over batch: B=64 split across 8 NeuronCores (8 batches/core).
All arithmetic runs on-device; the host only reshapes/transposes/shards and
casts the bool mask to f32.

Per-core layouts:
  phase 1: per-batch [D=128 partitions, T free] tiles.
  phase 2 state h.T: [128 partitions, 16] = two H-halves side by side.
  gi   (SBUF-resident fp16): [128, 6*T*8], free idx = c*(T*8) + t*8 + b
  decay(SBUF-resident fp16): [128, 2*T*8], free idx = h*(T*8) + t*8 + b
  out accum chunk (fp32):    [128, TC*16], free idx = t*16 + hh*8 + b
Recurrent matmuls run with fp16 weights/rhs (fp32 PSUM accumulate): fp32
matmuls on TRN2 lower to two LDWEIGHTS+MATMUL passes and the per-step cost
is LDWEIGHTS-bound, so 16-bit single-pass loads halve PE time; fp16 keeps
8x more mantissa than bf16 and |h|<=1, |w|<1 make the range safe.
"""

import numpy as np

import concourse.bass as bass
import concourse.mybir as mybir
import concourse.tile as tile

F32 = mybir.dt.float32
F16 = mybir.dt.float16
AF = mybir.ActivationFunctionType
OP = mybir.AluOpType

B = 8          # batches per core
D = 128
H = 256
GATE_CHUNKS = 6  # 3H/128
N_CORES = 8


def split_waits(nc, limit=1):
    """This walrus build accepts at most one sync wait per instruction;
    hoist extras onto single-wait NoOps preceding the instruction."""
    for bb in nc.main_func.blocks:
        new_insts = []
        for ins in bb.instructions:
            si = ins.sync_info
            waits = list(si.on_wait) if si is not None and si.on_wait else []
            if len(waits) > limit:
                extras, keep = waits[:-limit], waits[-limit:]
                for i, w in enumerate(extras):
                    nop = mybir.InstNoOp(name=f"{ins.name}-wsplit{i}", ins=[], outs=[])
                    nop.engine = ins.engine
                    nop.sync_info = mybir.SyncInfo(on_wait=[w], on_update=[])
                    nc.register_instruction(nop, overwrite=True)
                    new_insts.append(nop)
                si.on_wait = keep
            new_insts.append(ins)
        bb.instructions = new_insts


def build_nc(T=1024, TC=128, passes=1, do_pre=True, do_scan=True):
    assert T % TC == 0
    nc = bass.Bass()

    # ---- I/O ----
    xT = nc.dram_tensor("xT", [B, D, T], F32, kind="ExternalInput")
    maskT = nc.dram_tensor("maskT", [B, D, T], F32, kind="ExternalInput")
    tsteps = nc.dram_tensor("tsteps", [B, T], F32, kind="ExternalInput")
    tinit = nc.dram_tensor("tinit", [B, 1], F32, kind="ExternalInput")
    wihT = nc.dram_tensor("wihT", [D, 3 * H], F32, kind="ExternalInput")
    whhT = nc.dram_tensor("whhT", [H, 3 * H], F32, kind="ExternalInput")
    idwT = nc.dram_tensor("idwT", [D, D], F32, kind="ExternalInput")
    idb = nc.dram_tensor("idb", [D, 1], F32, kind="ExternalInput")
    b_rz_ih = nc.dram_tensor("b_rz_ih", [128, 4], F32, kind="ExternalInput")
    b_rz_hh = nc.dram_tensor("b_rz_hh", [128, 4], F32, kind="ExternalInput")
    b_n_ih = nc.dram_tensor("b_n_ih", [128, 2], F32, kind="ExternalInput")
    b_n_hh = nc.dram_tensor("b_n_hh", [128, 2], F32, kind="ExternalInput")
    hdw = nc.dram_tensor("hdw", [128, 2], F32, kind="ExternalInput")
    hdb = nc.dram_tensor("hdb", [128, 2], F32, kind="ExternalInput")

    hidden = nc.dram_tensor("hidden", [B, T, H], F32, kind="ExternalOutput")
    hlast = nc.dram_tensor("hlast", [B, H], F32, kind="ExternalOutput")

    td_scr = nc.dram_tensor("td_scr", [B, T], F32)
    tdl_scr = nc.dram_tensor("tdl_scr", [B, T], F32)

    with tile.TileContext(nc) as tc:
        with (
            tc.tile_pool(name="consts", bufs=1) as consts,
            tc.tile_pool(name="persist", bufs=1) as persist,
            tc.tile_pool(name="bwork", bufs=1) as bwork,
            tc.tile_pool(name="psum", bufs=2, space="PSUM") as psum,
            tc.tile_pool(name="psg", bufs=2, space="PSUM") as psg,
            tc.tile_pool(name="accch", bufs=2) as accch,
            tc.tile_pool(name="step", bufs=3) as steppool,
        ):
          # big SBUF-resident gate/decay stores (shared across passes)
          gi_all = persist.tile([128, GATE_CHUNKS * T * B], F16)
          dec_all = persist.tile([128, 2 * T * B], F16)
          gi_v = gi_all[:].rearrange("p (c t b) -> p c t b", c=GATE_CHUNKS, t=T)
          dec_v = dec_all[:].rearrange("p (h t b) -> p h t b", h=2, t=T)
          state0 = persist.tile([128, 16], F32)
          nc.vector.memset(state0[:], 0.0)

          for _pass in range(passes):
            # ---- constants / weights resident in SBUF ----
            ones128 = consts.tile([128, 1], F32)
            nc.vector.memset(ones128[:], 1.0)
            zeroT = consts.tile([128, T], F32)
            nc.vector.memset(zeroT[:], 0.0)

            wih_sb = consts.tile([128, 3 * H], F32)
            nc.sync.dma_start(wih_sb[:], wihT[:, :])
            whh0f = consts.tile([128, 3 * H], F32)
            nc.sync.dma_start(whh0f[:], whhT[0:128, :])
            whh1f = consts.tile([128, 3 * H], F32)
            nc.sync.dma_start(whh1f[:], whhT[128:256, :])
            whh0 = consts.tile([128, 3 * H], F16)
            nc.vector.tensor_copy(whh0[:], whh0f[:])
            whh1 = consts.tile([128, 3 * H], F16)
            nc.vector.tensor_copy(whh1[:], whh1f[:])
            idw_sb = consts.tile([128, D], F32)
            nc.sync.dma_start(idw_sb[:], idwT[:, :])

            idb_sb = consts.tile([128, 1], F32)
            nc.sync.dma_start(idb_sb[:], idb[:, :])
            brzi = consts.tile([128, 4], F32)
            nc.sync.dma_start(brzi[:], b_rz_ih[:, :])
            brzh = consts.tile([128, 4], F32)
            nc.sync.dma_start(brzh[:], b_rz_hh[:, :])
            bni = consts.tile([128, 2], F32)
            nc.sync.dma_start(bni[:], b_n_ih[:, :])
            bnh = consts.tile([128, 2], F32)
            nc.sync.dma_start(bnh[:], b_n_hh[:, :])
            hdw_sb = consts.tile([128, 2], F32)
            nc.sync.dma_start(hdw_sb[:], hdw[:, :])
            hdb_sb = consts.tile([128, 2], F32)
            nc.sync.dma_start(hdb_sb[:], hdb[:, :])

            brz = consts.tile([128, 4], F32)
            nc.vector.tensor_add(brz[:], brzi[:], brzh[:])
            nhdw = consts.tile([128, 2], F32)
            nc.vector.tensor_scalar_mul(nhdw[:], hdw_sb[:], -1.0)
            nhdb = consts.tile([128, 2], F32)
            nc.vector.tensor_scalar_mul(nhdb[:], hdb_sb[:], -1.0)
            # b_hh_n expanded to [128, 16] (half-major, 8 batches each)
            bhn_exp = consts.tile([128, 16], F32)
            for h in range(2):
                nc.vector.tensor_scalar_add(
                    bhn_exp[:, 8 * h : 8 * h + 8], zeroT[:, 0:8], bnh[:, h : h + 1]
                )
            # fw bias: pre = ic'@idwT + (idb - rowsum(idw)); exp bias = rowsum-idb
            ps_rs = psum.tile([128, 1], F32, tag="psfw")
            nc.tensor.matmul(ps_rs[:], idw_sb[:], ones128[:], start=True, stop=True)
            fw_ebias = consts.tile([128, 1], F32)
            nc.vector.tensor_tensor(
                out=fw_ebias[:], in0=ps_rs[:], in1=idb_sb[:], op=OP.subtract
            )

            # ---- stage A: time deltas ----
            ts8 = consts.tile([B, T + 1], F32)
            nc.sync.dma_start(ts8[:, 0:1], tinit[:, :])
            nc.sync.dma_start(ts8[:, 1 : T + 1], tsteps[:, :])
            td8 = consts.tile([B, T], F32)
            nc.vector.tensor_tensor(
                out=td8[:], in0=ts8[:, 1 : T + 1], in1=ts8[:, 0:T], op=OP.subtract
            )
            tdl8 = consts.tile([B, T], F32)
            nc.vector.tensor_scalar_min(tdl8[:], td8[:], 1000.0)
            nc.scalar.activation(tdl8[:], tdl8[:], AF.Ln)
            # spill so per-batch [128, T] broadcasts can use zero-stride
            # partition APs on the DMA source (~193 GB/s measured)
            nc.sync.dma_start(td_scr[:, :], td8[:])
            nc.sync.dma_start(tdl_scr[:, :], tdl8[:])

            # ---- phase 1 + gi per batch ----
            for b in range(B if do_pre else 0):
                tdbc = bwork.tile([128, T], F32, tag="tdbc")
                nc.sync.dma_start(tdbc[:], bass.AP(td_scr, b * T, [[0, 128], [1, T]]))
                tdlbc = bwork.tile([128, T], F32, tag="tdlbc")
                nc.sync.dma_start(tdlbc[:], bass.AP(tdl_scr, b * T, [[0, 128], [1, T]]))

                # hidden decay -> resident fp16 store (strided write)
                for h in range(2):
                    e = bwork.tile([128, T], F32, tag="dexp")
                    nc.scalar.activation(
                        e[:], tdlbc[:], AF.Exp,
                        bias=nhdb[:, h : h + 1], scale=nhdw[:, h : h + 1],
                    )
                    nc.vector.tensor_scalar_min(dec_v[:, h, :, b], e[:], 1.0)

                # mask tile with zero prefix column
                mT = bwork.tile([128, T + 1], F32, tag="mT")
                nc.vector.memset(mT[:, 0:1], 0.0)
                nc.sync.dma_start(mT[:, 1 : T + 1], maskT[b, :, :])
                xt = bwork.tile([128, T], F32, tag="xt")
                nc.sync.dma_start(xt[:], xT[b, :, :])

                # time-since-last-observation (pre-reset) scan:
                # v_t = m_{t-1} * v_{t-1} + td_t
                vt = bwork.tile([128, T], F32, tag="vt")
                nc.vector.tensor_tensor_scan(
                    vt[:], mT[:, 0:T], tdbc[:], 0.0, op0=OP.mult, op1=OP.add
                )
                # ic' = clip(v, 1, 1001); the -1 is folded into the fw bias
                ict = bwork.tile([128, T], F32, tag="dexp")
                nc.vector.tensor_scalar(
                    ict[:], vt[:], 1.0, 1001.0, op0=OP.max, op1=OP.min
                )
                # fill weight: fw = min(1, exp(-(ic'@idwT + idb - rowsum)))
                fwt = bwork.tile([128, T], F32, tag="fwt")
                nchunks = max(1, T // 512)
                csz = min(T, 512)
                for ck in range(nchunks):
                    ps_fw = psum.tile([128, csz], F32, tag="psfw")
                    nc.tensor.matmul(
                        ps_fw[:], idw_sb[:], ict[:, ck * csz : (ck + 1) * csz],
                        start=True, stop=True,
                    )
                    nc.scalar.activation(
                        fwt[:, ck * csz : (ck + 1) * csz], ps_fw[:], AF.Exp,
                        bias=fw_ebias[:, 0:1], scale=-1.0,
                    )
                nc.vector.tensor_scalar_min(fwt[:], fwt[:], 1.0)

                # xm = x where observed else 0
                xm = bwork.tile([128, T], F32, tag="tdbc")
                nc.vector.tensor_copy(xm[:], xt[:])
                nc.vector.copy_predicated(
                    xm[:], mT[:, 1 : T + 1].bitcast(mybir.dt.int32), zeroT[:]
                )
                # forward-fill scan: f_t = m_t * f_{t-1} + xm_t
                fft = bwork.tile([128, T], F32, tag="tdlbc")
                nc.vector.tensor_tensor_scan(
                    fft[:], mT[:, 1 : T + 1], xm[:], 0.0, op0=OP.mult, op1=OP.add
                )
                # mean = sum(xm) / max(1, sum(m))   (torch-faithful denominator)
                xsum = bwork.tile([128, 1], F32, tag="xsum")
                nc.vector.tensor_reduce(xsum[:], xm[:], axis=mybir.AxisListType.X, op=OP.add)
                msum = bwork.tile([128, 1], F32, tag="msum")
                nc.vector.tensor_reduce(
                    msum[:], mT[:, 1 : T + 1], axis=mybir.AxisListType.X, op=OP.add
                )
                nc.vector.tensor_scalar_max(msum[:], msum[:], 1.0)
                recd = bwork.tile([128, 1], F32, tag="recd")
                nc.vector.reciprocal(recd[:], msum[:])
                meant = bwork.tile([128, 1], F32, tag="meant")
                nc.vector.tensor_tensor(out=meant[:], in0=xsum[:], in1=recd[:], op=OP.mult)

                # filled = mean + fw*(ffill - mean); x_proc = where(m, filled, x)
                t1 = bwork.tile([128, T], F32, tag="vt")
                nc.vector.scalar_tensor_tensor(
                    out=t1[:], in0=fft[:], scalar=meant[:, 0:1], in1=fwt[:],
                    op0=OP.subtract, op1=OP.mult,
                )
                nc.scalar.activation(t1[:], t1[:], AF.Identity, bias=meant[:, 0:1])
                nc.vector.copy_predicated(
                    xt[:], mT[:, 1 : T + 1].bitcast(mybir.dt.int32), t1[:]
                )

                # gi = x_proc @ w_ih.T (+ folded biases) -> resident fp16 store
                for c in range(GATE_CHUNKS):
                    bias_ap = brz[:, c : c + 1] if c < 4 else bni[:, c - 4 : c - 3]
                    for ck in range(nchunks):
                        ps_gi = psum.tile([128, csz], F32, tag="psgi")
                        nc.tensor.matmul(
                            ps_gi[:],
                            wih_sb[:, 128 * c : 128 * (c + 1)],
                            xt[:, ck * csz : (ck + 1) * csz],
                            start=True, stop=True,
                        )
                        nc.scalar.activation(
                            gi_v[:, c, ck * csz : (ck + 1) * csz, b],
                            ps_gi[:], AF.Identity, bias=bias_ap,
                        )

            # ---- phase 2: the sequential scan ----
            n_chunks = (T // TC) if do_scan else 0
            state = state0[:]
            for k in range(n_chunks):
                acc = accch.tile([128, TC * 16], F32, tag="acc")
                for tt in range(TC):
                    t = k * TC + tt
                    # h' = decay_t * h_{t-1}
                    hp = steppool.tile([128, 16], F32, tag="hp")
                    nc.vector.tensor_tensor(
                        out=hp[:].rearrange("p (h b) -> p h b", h=2),
                        in0=state.rearrange("p (h b) -> p h b", h=2),
                        in1=dec_v[:, :, t, :], op=OP.mult,
                    )
                    hp_mm = steppool.tile([128, 16], F16, tag="hpb")
                    nc.vector.tensor_copy(hp_mm[:], hp[:])
                    # gh = h' @ w_hh.T  (12 fp16 matmuls, K=2x128, M=128, N=8)
                    ps_g = psg.tile([128, 48], F32, tag="psg")
                    for c in range(GATE_CHUNKS):
                        nc.tensor.matmul(
                            ps_g[:, 8 * c : 8 * c + 8],
                            whh0[:, 128 * c : 128 * (c + 1)],
                            hp_mm[:, 0:8],
                            start=True, stop=False,
                        )
                        nc.tensor.matmul(
                            ps_g[:, 8 * c : 8 * c + 8],
                            whh1[:, 128 * c : 128 * (c + 1)],
                            hp_mm[:, 8:16],
                            start=False, stop=True,
                        )
                    ps_s = psg.tile([128, 48], F32, tag="pss")
                    # pre_rz = gh_rz + gi_rz' (biases already folded into gi)
                    nc.vector.tensor_tensor(
                        out=ps_s[:, 0:32].rearrange("p (c b) -> p c b", c=4),
                        in0=ps_g[:, 0:32].rearrange("p (c b) -> p c b", c=4),
                        in1=gi_v[:, 0:4, t, :], op=OP.add,
                    )
                    rzt = steppool.tile([128, 32], F32, tag="rzt")
                    nc.scalar.activation(rzt[:], ps_s[:, 0:32], AF.Sigmoid)
                    # n = tanh(gi_n' + r*(gh_n + b_hh_n))
                    ut = steppool.tile([128, 16], F32, tag="ut")
                    nc.vector.tensor_tensor(
                        out=ut[:], in0=ps_g[:, 32:48], in1=bhn_exp[:], op=OP.add
                    )
                    u2 = steppool.tile([128, 16], F32, tag="u2")
                    nc.vector.tensor_tensor(out=u2[:], in0=ut[:], in1=rzt[:, 0:16], op=OP.mult)
                    nc.vector.tensor_tensor(
                        out=ps_s[:, 32:48].rearrange("p (c b) -> p c b", c=2),
                        in0=u2[:].rearrange("p (c b) -> p c b", c=2),
                        in1=gi_v[:, 4:6, t, :], op=OP.add,
                    )
                    nt = steppool.tile([128, 16], F32, tag="nt")
                    nc.scalar.activation(nt[:], ps_s[:, 32:48], AF.Tanh)
                    # h_t = n + z*(h' - n)
                    dt_ = steppool.tile([128, 16], F32, tag="dt")
                    nc.vector.tensor_tensor(out=dt_[:], in0=hp[:], in1=nt[:], op=OP.subtract)
                    et = steppool.tile([128, 16], F32, tag="et")
                    nc.vector.tensor_tensor(out=et[:], in0=rzt[:, 16:32], in1=dt_[:], op=OP.mult)
                    # h_t goes straight into the output accumulator; it is also
                    # the live state for step t+1.
                    state = acc[:, 16 * tt : 16 * tt + 16]
                    nc.vector.tensor_tensor(out=state, in0=nt[:], in1=et[:], op=OP.add)

                acc_r = acc[:].rearrange("p (t hh b) -> p t hh b", t=TC, hh=2)
                for b in range(B):
                    nc.sync.dma_start(
                        hidden[b, k * TC : (k + 1) * TC, :].rearrange(
                            "t (hh p) -> p t hh", p=128
                        ),
                        acc_r[:, :, :, b],
                    )

            if do_scan:
                state_r = state.rearrange("p (hh b) -> p hh b", hh=2)
                for b in range(B):
                    nc.sync.dma_start(
                        hlast[b, :].rearrange("(hh p) -> p hh", p=128),
                        state_r[:, :, b],
                    )

    split_waits(nc)
    return nc


def _host_prepare(inputs, T):
    """Pure layout transforms; no arithmetic beyond dtype cast of the bool mask."""
    x = np.asarray(inputs["input"], np.float32)
    mask = np.asarray(inputs["mask"]).astype(np.float32)
    time_step = np.asarray(inputs["time_step"], np.float32)
    init_time = np.asarray(inputs["init_time"], np.float32)
    w_ih = np.asarray(inputs["w_ih"], np.float32)
    w_hh = np.asarray(inputs["w_hh"], np.float32)
    b_ih = np.asarray(inputs["b_ih"], np.float32)
    b_hh = np.asarray(inputs["b_hh"], np.float32)
    idw = np.asarray(inputs["input_decay_w"], np.float32)
    idb = np.asarray(inputs["input_decay_b"], np.float32)
    hdw = np.asarray(inputs["hidden_decay_w"], np.float32)
    hdb = np.asarray(inputs["hidden_decay_b"], np.float32)

    shared = dict(
        wihT=np.ascontiguousarray(w_ih.T),
        whhT=np.ascontiguousarray(w_hh.T),
        idwT=np.ascontiguousarray(idw.T),
        idb=np.ascontiguousarray(idb.reshape(1, D).T),
        b_rz_ih=np.ascontiguousarray(b_ih[: 2 * H].reshape(4, 128).T),
        b_rz_hh=np.ascontiguousarray(b_hh[: 2 * H].reshape(4, 128).T),
        b_n_ih=np.ascontiguousarray(b_ih[2 * H :].reshape(2, 128).T),
        b_n_hh=np.ascontiguousarray(b_hh[2 * H :].reshape(2, 128).T),
        hdw=np.ascontiguousarray(hdw[:, 0].reshape(2, 128).T),
        hdb=np.ascontiguousarray(hdb.reshape(2, 128).T),
    )
    in_maps = []
    for c in range(N_CORES):
        sl = slice(c * B, (c + 1) * B)
        m = dict(shared)
        m["xT"] = np.ascontiguousarray(x[sl].transpose(0, 2, 1))
        m["maskT"] = np.ascontiguousarray(mask[sl].transpose(0, 2, 1))
        m["tsteps"] = np.ascontiguousarray(time_step[sl])
        m["tinit"] = np.ascontiguousarray(init_time[0, sl].reshape(B, 1))
        in_maps.append(m)
    return in_maps


def kernel(**inputs):
    from concourse.bass_utils import run_bass_kernel_spmd

    T = inputs["input"].shape[1]
    nc = build_nc(T=T, TC=min(128, T))
    in_maps = _host_prepare(inputs, T)
    res = run_bass_kernel_spmd(nc, in_maps, core_ids=list(range(N_CORES)))
    hidden = np.concatenate([r["hidden"] for r in res.results], axis=0)
    hlast = np.concatenate([r["hlast"] for r in res.results], axis=0)
    return hidden, hlast[None]
